# revision 2
# baseline (speedup 1.0000x reference)
"""CAM-GAT layer kernel for 8 Trainium2 NeuronCores (Bass/Tile) — v2.

Reference math (per graph of N=21 joints, F=128 feats):
    h = x @ W1                         [N, F]
    s = h @ a1 ; t = h @ a2            [N]
    e[i,j] = leaky_relu(s_i + t_j, 0.2)
    beta = softmax_j(e)
    alpha = cam * beta
    x_agg = alpha @ h
    out = elu(concat([x_agg, x], -1) @ W2_w + W2_b)

Key algebra: x_agg @ W2a = alpha @ (x @ (W1 @ W2a)) = alpha @ g, so h is
never materialized; g = x @ W12a with W12a precomputed on the host.

Sharding: pure data parallelism; each core gets B/8 = 2048 graphs
(43008 rows); weights replicated.

Per-core dataflow (supertile = 504 rows = 4 chunks x 126 rows = 24 graphs):
  xT    : PE transpose of fp32 x chunks; cast to bf16 in the PSUM->SBUF copy
  s,t   : one matmul [wa1|wa2]^T @ xT -> st [2, 504]
  e_cmp : compact attention [126, (c, jj)] = [126, 4, 21]; one matmul with
          L rows = dyn s + graph indicators, R rows = chunk delta + t-reshape
  smax  : Prelu(0.2) -> Exp (compact) -> DVE row-reduce -> reciprocal ->
          beta_cmp = E * rinv (tensor_scalar per chunk)
  at    : PE transpose beta_cmp -> [84, 126]; PE spread matmul to
          [126(j), c, 126(i)]; gate+cam via one TT against static camT
  o     : per chunk: bias (K=1 mm) + at^T @ g + xT^T @ W2b in one PSUM
  elu   : em=Exp(o), r=Relu(o) on ACT; out = min(em-1, r) on GpSimd
"""

import sys

import numpy as np

try:
    import concourse  # noqa: F401
except ImportError:  # pragma: no cover
    sys.path.insert(0, "/opt/trn_rl_repo")

import ml_dtypes
import concourse.bass as bass
import concourse.bacc as bacc
import concourse.tile as tile
from concourse import mybir

FP32 = mybir.dt.float32
BF16 = mybir.dt.bfloat16
AF = mybir.ActivationFunctionType
ALU = mybir.AluOpType

N_JOINTS = 21
F = 128
B_TOTAL = 16384
N_CORES = 8
B_CORE = B_TOTAL // N_CORES            # 2048 graphs per core
ROWS_CORE = B_CORE * N_JOINTS          # 43008 rows per core

G_CHUNK = 6                            # graphs per chunk
RC = G_CHUNK * N_JOINTS                # 126 rows per chunk
NCH = 4                                # chunks per (full) supertile
ROWS_SUPER = NCH * RC                  # 504
ST_SLAB = 8                            # supertiles per DMA slab
ROWS_SLAB = ST_SLAB * ROWS_SUPER       # 4032
CH_SLAB = ST_SLAB * NCH                # 32 chunk slots per slab
NCHT = (ROWS_CORE + RC - 1) // RC + 1  # 342 chunks/core (padded, even)


def _slab_plan(ncht):
    """[[chunks-per-supertile...] per slab] over ncht padded chunks."""
    plan = []
    c = 0
    while c < ncht:
        ns = min(CH_SLAB, ncht - c)
        sts = []
        k = 0
        while k < ns:
            sts.append(min(NCH, ns - k))
            k += NCH
        plan.append(sts)
        c += ns
    return plan


def host_consts(cam, W1, a, W2_w, W2_b):
    """Precompute tiny replicated tensors on the host (numpy)."""
    cam = np.asarray(cam, np.float32)
    W1 = np.asarray(W1, np.float32)
    a = np.asarray(a, np.float32)
    W2_w = np.asarray(W2_w, np.float32)
    W2_b = np.asarray(W2_b, np.float32)
    bf = ml_dtypes.bfloat16

    W12a = W1 @ W2_w[:F]                     # [128,128] g-space weight
    wa12 = np.stack([W1 @ a[:F], W1 @ a[F:]], axis=1)  # [128, 2]

    ident_f = np.eye(RC, dtype=np.float32)
    ident_b = ident_f.astype(bf)

    blk = np.arange(RC) // N_JOINTS

    # e_s matmul rhs: wa1 broadcast over the 21 mate columns
    wa1ones = np.tile((W1 @ a[:F])[:, None], (1, N_JOINTS))  # [128, 21]
    # e_t stationary: lind[q, i] = ind(i//21 == q), padded to 128 cols
    lind = np.zeros((G_CHUNK, F), np.float32)
    for q in range(G_CHUNK):
        lind[q, :RC] = (blk == q)

    # spread stationaries SP_c [84, 128]: SP[(c',jj), j] = d(c'==c)d(jj==j%21)
    SP = np.zeros((NCH, NCH * N_JOINTS, F), np.float32)
    for c in range(NCH):
        for j in range(RC):
            SP[c, c * N_JOINTS + (j % N_JOINTS), j] = 1.0

    # camT[j, i] = cam[i%21, j%21] * (i//21 == j//21)  (gate + cam in one)
    camT = np.zeros((RC, RC), np.float32)
    for q in range(G_CHUNK):
        s0 = q * N_JOINTS
        camT[s0:s0 + N_JOINTS, s0:s0 + N_JOINTS] = cam.T

    atpad = np.zeros((2, NCH, F), np.float32)
    atpad[0] = 1.0

    return {
        "atpad": atpad.astype(bf),               # [2,4,128]
        "w12a": W12a.astype(bf),                 # [128,128]
        "w2bb": W2_w[F:].astype(bf),             # [128,128]
        "wa2c": wa12[:, 1:2].astype(bf),         # [128,1]
        "w2brow": W2_b.reshape(1, F).astype(bf),  # [1,128]
        "identf": ident_f,                       # [126,126] f32
        "identb": ident_b,                       # [126,126] bf16
        "wa1ones": wa1ones.astype(bf),           # [128,21]
        "lind": lind.astype(bf),                 # [6,128]
        "sp": SP.astype(bf),                     # [4,84,128]
        "camt": camT.astype(bf),                 # [126,126]
    }


CONST_SPECS = {
    "atpad": ([2, NCH, F], BF16),
    "w12a": ([F, F], BF16),
    "w2bb": ([F, F], BF16),
    "wa2c": ([F, 1], BF16),
    "w2brow": ([1, F], BF16),
    "identf": ([RC, RC], FP32),
    "identb": ([RC, RC], BF16),
    "wa1ones": ([F, N_JOINTS], BF16),
    "lind": ([G_CHUNK, F], BF16),
    "sp": ([NCH, NCH * N_JOINTS, F], BF16),
    "camt": ([RC, RC], BF16),
}


def build_program(rows=ROWS_CORE):
    nc = bacc.Bacc("TRN2", target_bir_lowering=False, debug=False,
                   enable_asserts=False)
    x_d = nc.dram_tensor("x", [rows, F], FP32, kind="ExternalInput").ap()
    out_d = nc.dram_tensor("out", [rows, F], FP32, kind="ExternalOutput").ap()
    cst = {k: nc.dram_tensor(k, shape, dt, kind="ExternalInput").ap()
           for k, (shape, dt) in CONST_SPECS.items()}
    with tile.TileContext(nc) as tc:
        _body(tc, x_d, out_d, cst, rows)
    nc.compile()
    return nc


def _bcast_c(ap, n):
    """Insert a stride-0 dim after the partition dim: [P, X] -> [P, n, X]."""
    p, rest = ap.ap[0], list(ap.ap[1:])
    assert len(rest) == 1
    return bass.AP(ap.tensor, ap.offset, [p, [0, n], rest[0]])


def _perm_qcj(xt_sl, c0):
    """View xt_sl chunks [c0, c0+4) [F, c, j=21q+jj] as [F, (q, c, jj)]."""
    ap = xt_sl[:, c0:c0 + NCH, 0:RC]
    return bass.AP(ap.tensor, ap.offset,
                   [ap.ap[0], [N_JOINTS, G_CHUNK], [F, NCH], [1, N_JOINTS]])


def _body(tc, x_d, out_d, cst, rows):
    from contextlib import ExitStack
    nc = tc.nc
    plan = _slab_plan(rows)

    with ExitStack() as ctx:
        # ---- pools ----
        cpool = ctx.enter_context(tc.tile_pool(name="consts", bufs=1))
        pxin = ctx.enter_context(tc.tile_pool(name="xslab", bufs=2))
        pout = ctx.enter_context(tc.tile_pool(name="oslab", bufs=2))
        pxt = ctx.enter_context(tc.tile_pool(name="xt", bufs=2))
        pst = ctx.enter_context(tc.tile_pool(name="stsb", bufs=2))
        pu = ctx.enter_context(tc.tile_pool(name="ucmp", bufs=3))
        pe_ = ctx.enter_context(tc.tile_pool(name="ecmp", bufs=3))
        psc = ctx.enter_context(tc.tile_pool(name="scal", bufs=3))
        pac = ctx.enter_context(tc.tile_pool(name="acmp", bufs=3))
        pat2 = ctx.enter_context(tc.tile_pool(name="atc", bufs=3))
        pat = ctx.enter_context(tc.tile_pool(name="atbd", bufs=3))
        pg = ctx.enter_context(tc.tile_pool(name="gsb", bufs=3))
        pem = ctx.enter_context(tc.tile_pool(name="embuf", bufs=3))
        pr = ctx.enter_context(tc.tile_pool(name="rbuf", bufs=3))
        pr3 = ctx.enter_context(tc.tile_pool(name="r3", bufs=2))
        pxb = ctx.enter_context(tc.tile_pool(name="xbf", bufs=2))

        ps_xt = ctx.enter_context(tc.tile_pool(name="ps_xt", bufs=1, space="PSUM"))
        ps_st = ctx.enter_context(tc.tile_pool(name="ps_st", bufs=1, space="PSUM"))
        ps_e = ctx.enter_context(tc.tile_pool(name="ps_e", bufs=1, space="PSUM"))
        ps_at1 = ctx.enter_context(tc.tile_pool(name="ps_at1", bufs=1, space="PSUM"))
        ps_at2 = ctx.enter_context(tc.tile_pool(name="ps_at2", bufs=1, space="PSUM"))
        ps_g = ctx.enter_context(tc.tile_pool(name="ps_g", bufs=1, space="PSUM"))
        ps_o = ctx.enter_context(tc.tile_pool(name="ps_o", bufs=2, space="PSUM"))

        # ---- load constants ----
        w12a = cpool.tile([F, F], BF16, tag="w12a")
        w2bb = cpool.tile([F, F], BF16, tag="w2bb")
        wa2c = cpool.tile([F, 1], BF16, tag="wa2c")
        w2brow = cpool.tile([1, F], BF16, tag="w2brow")
        identf = cpool.tile([RC, RC], FP32, tag="identf")
        identb = cpool.tile([RC, RC], BF16, tag="identb")
        wa1ones = cpool.tile([F, N_JOINTS], BF16, tag="wa1ones")
        lind = cpool.tile([G_CHUNK, F], BF16, tag="lind")
        sp = cpool.tile([NCH * N_JOINTS, NCH, F], BF16, tag="sp")
        camt = cpool.tile([RC, RC], BF16, tag="camt")
        for name, t in (("w12a", w12a), ("w2bb", w2bb), ("wa2c", wa2c),
                        ("w2brow", w2brow), ("identf", identf),
                        ("identb", identb), ("wa1ones", wa1ones),
                        ("lind", lind), ("camt", camt)):
            nc.sync.dma_start(t[:], cst[name][:])
        nc.sync.dma_start(sp[:], cst["sp"].rearrange("c k f -> k c f"))

        r0 = 0
        sli = 0
        sti = 0

        def setup_slab(slab_rows, sts, r0, sli):
            nfull = slab_rows // RC
            rem = slab_rows - nfull * RC
            nst = len(sts)
            x_sl = pxin.tile([RC, CH_SLAB, F], FP32, tag="x_sl")
            x_bf = pxb.tile([RC, CH_SLAB, F], BF16, tag="x_bf")
            o_sl = pout.tile([RC, CH_SLAB, F], FP32, tag="o_sl")
            if nfull:
                nc.sync.dma_start(
                    x_sl[:, 0:nfull, :],
                    x_d[r0:r0 + nfull * RC, :].rearrange(
                        "(c i) f -> i c f", i=RC))
            if rem:
                nc.gpsimd.memset(x_sl[:, nfull, :], 0.0)
                nc.sync.dma_start(
                    x_sl[0:rem, nfull, :],
                    x_d[r0 + nfull * RC:r0 + slab_rows, :])
            xt_sl = pxt.tile([F, CH_SLAB, F], BF16, tag="xt_sl")
            st_sb = pst.tile([1, G_CHUNK, CH_SLAB, N_JOINTS], BF16,
                             tag="st_sb")
            R3 = pr3.tile([G_CHUNK, CH_SLAB, N_JOINTS], BF16, tag="R3")
            if sli <= 2:
                nc.gpsimd.memset(xt_sl[:, :, RC:F], 0.0)
            if nst * NCH < CH_SLAB:
                nc.gpsimd.memset(xt_sl[:, nst * NCH:CH_SLAB, 0:RC], 0.0)
                nc.gpsimd.memset(st_sb[:, :, nst * NCH:CH_SLAB, :], 0.0)
            return dict(sts=sts, r0=r0, nfull=nfull, rem=rem, nst=nst,
                        x_sl=x_sl, x_bf=x_bf, o_sl=o_sl, xt_sl=xt_sl,
                        st_sb=st_sb, R3=R3)

        def emit_pass1(sl, s):
            chunks = sl["sts"][s]
            nch = len(chunks)
            c0 = s * NCH
            x_sl, x_bf, xt_sl = sl["x_sl"], sl["x_bf"], sl["xt_sl"]
            # cast to bf16 on GpSimd, then cheap bf16 PE transposes
            nc.gpsimd.tensor_copy(x_bf[:, c0:c0 + nch, :],
                                  x_sl[:, c0:c0 + nch, :])
            xt_ps = ps_xt.tile([F, NCH, F], BF16, tag="xt_ps")
            for c in range(nch):
                nc.tensor.transpose(xt_ps[:, c, 0:RC],
                                    x_bf[:, c0 + c, :], identb[:])
            if nch < NCH:
                nc.gpsimd.memset(xt_sl[:, c0 + nch:c0 + NCH, 0:RC], 0.0)
            nh = (nch + 1) // 2
            nc.vector.tensor_copy(xt_sl[:, c0:c0 + nh, 0:RC],
                                  xt_ps[:, 0:nh, 0:RC])
            if nch > nh:
                nc.scalar.copy(xt_sl[:, c0 + nh:c0 + nch, 0:RC],
                               xt_ps[:, nh:nch, 0:RC])
            # t in (q, c, jj) order: t[mate] = wa2 . x_row
            st_ps = ps_st.tile([1, G_CHUNK, NCH, N_JOINTS], FP32,
                               tag="st_ps")
            nc.tensor.matmul(st_ps[:], wa2c[:], _perm_qcj(xt_sl, c0),
                             start=True, stop=True)
            nc.vector.tensor_copy(sl["st_sb"][:, :, c0:c0 + NCH, :],
                                  st_ps[:])

        def emit_scatter(sl):
            # one partition-scatter per slab: [1, (q, c, jj)] -> [q, c, jj]
            flat = sl["st_sb"][:]
            nc.gpsimd.dma_start(
                sl["R3"][:],
                bass.AP(flat.tensor, flat.offset,
                        [flat.ap[0], [1, G_CHUNK * CH_SLAB * N_JOINTS]]))

        def emit_pass2(sl, s, sti):
            chunks = sl["sts"][s]
            nch = len(chunks)
            c0 = s * NCH
            xt_sl, R3, o_sl = sl["xt_sl"], sl["R3"], sl["o_sl"]

            # -- compact e: e[i, (c, jj)] = s_i + t[mate] --
            e_ps = ps_e.tile([F, NCH, N_JOINTS], FP32, tag="e_ps")
            for c in range(nch):
                nc.tensor.matmul(e_ps[:, c, :], xt_sl[:, c0 + c, :],
                                 wa1ones[:], start=True, stop=False)
                nc.tensor.matmul(e_ps[:, c, :], lind[:], R3[:, c0 + c, :],
                                 start=False, stop=True)

            # -- softmax (compact): prelu -> exp -> reduce -> recip --
            u = pu.tile([RC, NCH, N_JOINTS], FP32, tag="u")
            nc.scalar.activation(u[:, 0:nch, :], e_ps[0:RC, 0:nch, :],
                                 AF.Prelu, alpha=0.2)
            E = pe_.tile([RC, NCH, N_JOINTS], BF16, tag="E")
            nc.scalar.activation(E[:, 0:nch, :], u[:, 0:nch, :], AF.Exp)
            rowsum = psc.tile([RC, NCH], FP32, tag="rowsum")
            nc.vector.tensor_reduce(rowsum[:, 0:nch], E[:, 0:nch, :],
                                    mybir.AxisListType.X, ALU.add)
            rinv = psc.tile([RC, NCH], FP32, tag="rinv")
            nc.vector.reciprocal(rinv[:, 0:nch], rowsum[:, 0:nch])

            # -- beta_cmp = E * rinv --
            acmp = pac.tile([RC, NCH, N_JOINTS], BF16, tag="acmp")
            for c in range(nch):
                nc.vector.tensor_scalar(
                    acmp[:, c, :], E[:, c, :], rinv[:, c:c + 1], None,
                    ALU.mult)

            # -- transpose compact beta: [84, 126] --
            at1_ps = ps_at1.tile([NCH * N_JOINTS, F], BF16, tag="at1_ps")
            nc.tensor.transpose(at1_ps[0:nch * N_JOINTS, 0:RC],
                                acmp[:, 0:nch, :], identb[:])
            atc = pat2.tile([NCH * N_JOINTS, RC], BF16, tag="atc")
            nc.vector.tensor_copy(atc[0:nch * N_JOINTS, :],
                                  at1_ps[0:nch * N_JOINTS, 0:RC])

            # -- spread to block-diag (ungated), then gate*cam --
            at2_ps = ps_at2.tile([F, NCH, RC], FP32, tag="at2_ps")
            for c in range(nch):
                nc.tensor.matmul(at2_ps[:, c, :],
                                 sp[0:nch * N_JOINTS, c, :],
                                 atc[0:nch * N_JOINTS, :],
                                 start=True, stop=True)
            at = pat.tile([F, NCH, F], BF16, tag="at")
            if sti <= 3:
                # one-time: junk cols 0, bias row (126) = ones, row 127 = 0
                nc.gpsimd.memset(at[0:RC, :, RC:F], 0.0)
                nc.gpsimd.dma_start(at[RC:F, :, :], cst["atpad"][:])
            nc.vector.tensor_tensor(
                at[0:RC, 0:nch, 0:RC], at2_ps[0:RC, 0:nch, :],
                _bcast_c(camt[:], nch), ALU.mult)

            # -- g = x @ W12a (row-major, bf16); row 126 = W2_b --
            g_ps = ps_g.tile([F, NCH, F], FP32, tag="g_ps")
            for c in range(nch):
                nc.tensor.matmul(g_ps[:, c, :], xt_sl[:, c0 + c, :],
                                 w12a[:], start=True, stop=True)
            g = pg.tile([F, NCH, F], BF16, tag="g")
            if sti <= 3:
                nc.sync.dma_start(g[RC:RC + 1, :, :],
                                  _bcast_c(cst["w2brow"][:], NCH))
            nc.scalar.copy(g[0:RC, 0:nch, :], g_ps[0:RC, 0:nch, :])

            # -- o = at^T @ [g; W2_b] + x @ W2b  (bias via K=127) --
            o_ps = ps_o.tile([F, NCH, F], FP32, tag="o_ps")
            for c in range(nch):
                nc.tensor.matmul(o_ps[:, c, :], at[0:RC + 1, c, :],
                                 g[0:RC + 1, c, :], start=True, stop=False)
                nc.tensor.matmul(o_ps[:, c, :], xt_sl[:, c0 + c, :],
                                 w2bb[:], start=False, stop=True)

            # -- elu: em=exp(o), r=relu(o); out = min(em-1, r) --
            em = pem.tile([RC, NCH, F], BF16, tag="em")
            nc.scalar.activation(em[:, 0:nch, :], o_ps[0:RC, 0:nch, :],
                                 AF.Exp)
            rr = pr.tile([RC, NCH, F], BF16, tag="rr")
            nc.scalar.activation(rr[:, 0:nch, :], o_ps[0:RC, 0:nch, :],
                                 AF.Relu)
            nc.vector.scalar_tensor_tensor(
                o_sl[:, c0:c0 + nch, :], em[:, 0:nch, :], -1.0,
                rr[:, 0:nch, :], op0=ALU.add, op1=ALU.min)

        def emit_store(sl):
            nfull, rem, r0s, o_sl = sl["nfull"], sl["rem"], sl["r0"], sl["o_sl"]
            if nfull:
                nc.sync.dma_start(
                    out_d[r0s:r0s + nfull * RC, :].rearrange(
                        "(c i) f -> i c f", i=RC),
                    o_sl[:, 0:nfull, :])
            if rem:
                nc.sync.dma_start(
                    out_d[r0s + nfull * RC:r0s + nfull * RC + rem, :],
                    o_sl[0:rem, nfull, :])

        prev = None
        for slab_rows, sts in plan:
            sli += 1
            cur = setup_slab(slab_rows, sts, r0, sli)
            r0 += slab_rows
            nmax = max(cur["nst"], prev["nst"] if prev else 0)
            for s in range(nmax):
                if s < cur["nst"]:
                    emit_pass1(cur, s)
                if prev and s < prev["nst"]:
                    sti += 1
                    emit_pass2(prev, s, sti)
            emit_scatter(cur)
            if prev:
                emit_store(prev)
            prev = cur
        for s in range(prev["nst"]):
            sti += 1
            emit_pass2(prev, s, sti)
        emit_store(prev)


# ---------------------------------------------------------------------------
_PROG_CACHE = {}


def _get_program(rows):
    if rows not in _PROG_CACHE:
        _PROG_CACHE[rows] = build_program(rows)
    return _PROG_CACHE[rows]


def kernel(x, cam, W1, a, W2_w, W2_b):
    from concourse.bass_utils import run_bass_kernel_spmd

    x = np.ascontiguousarray(np.asarray(x, np.float32))
    consts = host_consts(cam, W1, a, W2_w, W2_b)
    nc = _get_program(ROWS_CORE)

    in_maps = []
    for core in range(N_CORES):
        m = {"x": x[core * ROWS_CORE:(core + 1) * ROWS_CORE]}
        m.update(consts)
        in_maps.append(m)
    res = run_bass_kernel_spmd(nc, in_maps, list(range(N_CORES)))
    out = np.concatenate([res.results[i]["out"] for i in range(N_CORES)], axis=0)
    return out.astype(np.float32)



# revision 20
# speedup vs baseline: 1.6171x; 1.6171x over previous
"""CAM-GAT layer kernel for 8 Trainium2 NeuronCores (Bass/Tile) — v3.

Reference math (per graph of N=21 joints, F=128 feats):
    h = x @ W1                         [N, F]
    s = h @ a1 ; t = h @ a2            [N]
    e[i,j] = leaky_relu(s_i + t_j, 0.2)
    beta = softmax_j(e)
    alpha = cam * beta
    x_agg = alpha @ h
    out = elu(concat([x_agg, x], -1) @ W2_w + W2_b)

Key algebra: x_agg @ W2a = alpha @ (x @ (W1 @ W2a)) = alpha @ g with
g = x @ W12a (W12a host-precomputed); h never materialized.

v3 layout strategy (vs v2): the host supplies x TRANSPOSED, padded and
chunk-blocked as xt[128 f, ncht, 126] bf16, and receives the output
back f-major as out[128 f, ncht, 126] bf16.  This removes all on-device
PE transposes, the fp32->bf16 cast and the transpose-copy traffic, and
halves HBM traffic.  All dataflow is feature-major:

  per chunk (126 rows = 6 graphs):
    e_ps[i, 0:21|21] = xt_c^T @ [wa1*ones | wa2]   (s_i bcast, t fused)
    g_ps[j, f]       = xt_c^T @ W12a               (row-major g)
  per supertile (8 chunks = 1008 rows):
    t scatter        : gpsimd dma e_ps[:,:,21] -> R3[6, 21, 8]
    e_ps[:, :, 0:21] += lind^T @ R3                (one matmul, all chunks)
    prelu -> (slab-wide) exp -> rowsum -> recip
    acmp = E * rinv * camrep                       (cam folded into compact)
    at2_ps[j, c, i]  = spread-view(acmp)^T via PE transpose  (bf16 PSUM)
    at[j, c, i]      = at2 * blockdiag01           (gate only)
    o_ps[f, c, i]    = W2b^T @ xt (1 mm / 4 chunks) + g_c^T @ at_c
    elu: em=Exp(o), rr=Relu(o), out = min(em-1, rr)
"""

import sys

import numpy as np

try:
    import concourse  # noqa: F401
except ImportError:  # pragma: no cover
    sys.path.insert(0, "/opt/trn_rl_repo")

import ml_dtypes
import concourse.bass as bass
import concourse.bacc as bacc
import concourse.tile as tile
from concourse import mybir

FP32 = mybir.dt.float32
BF16 = mybir.dt.bfloat16
AF = mybir.ActivationFunctionType
ALU = mybir.AluOpType

N_JOINTS = 21
F = 128
B_TOTAL = 16384
N_CORES = 8
B_CORE = B_TOTAL // N_CORES            # 2048 graphs per core
ROWS_CORE = B_CORE * N_JOINTS          # 43008 real rows per core

G_CHUNK = 6                            # graphs per chunk
RC = G_CHUNK * N_JOINTS                # 126 rows per chunk
NCH2 = 8                               # chunks per supertile
CH_SLAB = 32                           # chunks per DMA slab
NCHT = -(-ROWS_CORE // RC)             # 342 padded chunks per core
ROWS_PAD = NCHT * RC                   # 43092


def _slab_plan(ncht):
    """[[chunks-per-supertile...] per slab] over ncht padded chunks."""
    plan = []
    c = 0
    while c < ncht:
        ns = min(CH_SLAB, ncht - c)
        sts = []
        k = 0
        while k < ns:
            sts.append(min(NCH2, ns - k))
            k += NCH2
        plan.append(sts)
        c += ns
    return plan


def host_consts(cam, W1, a, W2_w, W2_b):
    """Precompute tiny replicated tensors on the host (numpy)."""
    cam = np.asarray(cam, np.float32)
    W1 = np.asarray(W1, np.float32)
    a = np.asarray(a, np.float32)
    W2_w = np.asarray(W2_w, np.float32)
    W2_b = np.asarray(W2_b, np.float32)
    bf = ml_dtypes.bfloat16

    wa1 = W1 @ a[:F]
    wa2 = W1 @ a[F:]
    W12a = W1 @ W2_w[:F]

    # e matmul moving: cols 0..20 = wa1 (s bcast over mates), col 21 = wa2 (t)
    esmov = np.concatenate(
        [np.tile(wa1[:, None], (1, N_JOINTS)), wa2[:, None]], axis=1)

    blk = np.arange(RC) // N_JOINTS
    lind = (blk[None, :] == np.arange(G_CHUNK)[:, None]).astype(np.float32)

    camt01 = (blk[:, None] == blk[None, :]).astype(bf)     # block-diag gate
    camrep = np.tile(cam, (G_CHUNK, 1))                    # [126,21] cam[i%21,jj]

    # spread stationaries: SP[(c',jj), c, j] = d(c'==c) d(jj==j%21)
    SP = np.zeros((4 * N_JOINTS, 4, RC), np.float32)
    for c in range(4):
        for j in range(RC):
            SP[c * N_JOINTS + (j % N_JOINTS), c, j] = 1.0

    return {
        "esmov": esmov.astype(bf),               # [128,22]
        "lindf": lind.astype(bf),                # [6,126]
        "identb": np.eye(RC, dtype=np.float32).astype(bf),  # [126,126]
        "camt01": camt01,                        # [126,126]
        "camrep": camrep.astype(bf),             # [126,21]
        "w12a": W12a.astype(bf),                 # [128,128]
        "w2bb": W2_w[F:].astype(bf),             # [128,128]
        "w2brow": W2_b.reshape(1, F).astype(bf),  # [1,128]
        "sp": SP.astype(bf),                     # [84,4,128]
        "ones126": np.ones((1, RC), np.float32).astype(bf),  # [1,126]
    }


CONST_SPECS = {
    "esmov": ([F, N_JOINTS + 1], BF16),
    "lindf": ([G_CHUNK, RC], BF16),
    "identb": ([RC, RC], BF16),
    "camt01": ([RC, RC], BF16),
    "camrep": ([RC, N_JOINTS], BF16),
    "w12a": ([F, F], BF16),
    "w2bb": ([F, F], BF16),
    "w2brow": ([1, F], BF16),
    "sp": ([4 * N_JOINTS, 4, RC], BF16),
    "ones126": ([1, RC], BF16),
}


def build_program(ncht=NCHT):
    nc = bacc.Bacc("TRN2", target_bir_lowering=False, debug=False,
                   enable_asserts=False)
    x_d = nc.dram_tensor("x", [F, ncht, RC], BF16, kind="ExternalInput").ap()
    out_d = nc.dram_tensor("out", [F, ncht, RC], BF16,
                           kind="ExternalOutput").ap()
    cst = {k: nc.dram_tensor(k, shape, dt, kind="ExternalInput").ap()
           for k, (shape, dt) in CONST_SPECS.items()}
    with tile.TileContext(nc) as tc:
        _body(tc, x_d, out_d, cst, ncht)
    nc.compile()
    return nc


def _view(ap, extra_off, dims):
    """Raw AP view: same tensor, offset bumped, free dims replaced."""
    return bass.AP(ap.tensor, ap.offset + extra_off, [ap.ap[0]] + dims)


def _body(tc, x_d, out_d, cst, ncht):
    from contextlib import ExitStack
    nc = tc.nc
    plan = _slab_plan(ncht)

    with ExitStack() as ctx:
        # ---- pools ----
        cpool = ctx.enter_context(tc.tile_pool(name="consts", bufs=1))
        pxt = ctx.enter_context(tc.tile_pool(name="xt", bufs=2))
        pout = ctx.enter_context(tc.tile_pool(name="osl", bufs=2))
        pu = ctx.enter_context(tc.tile_pool(name="u", bufs=2))
        pe_ = ctx.enter_context(tc.tile_pool(name="E", bufs=2))
        prs = ctx.enter_context(tc.tile_pool(name="rsum", bufs=2))
        pri = ctx.enter_context(tc.tile_pool(name="rinv", bufs=2))
        prc = ctx.enter_context(tc.tile_pool(name="rcam", bufs=2))
        pac = ctx.enter_context(tc.tile_pool(name="acmp", bufs=2))
        pr3 = ctx.enter_context(tc.tile_pool(name="r3", bufs=2))
        pts = ctx.enter_context(tc.tile_pool(name="tsb", bufs=2))
        pat = ctx.enter_context(tc.tile_pool(name="at", bufs=2))
        patc = ctx.enter_context(tc.tile_pool(name="atc", bufs=2))
        pg = ctx.enter_context(tc.tile_pool(name="gsb", bufs=5))
        pem = ctx.enter_context(tc.tile_pool(name="em", bufs=2))
        prr = ctx.enter_context(tc.tile_pool(name="rr", bufs=2))

        ps_e = ctx.enter_context(tc.tile_pool(name="ps_e", bufs=2,
                                              space="PSUM"))
        ps_g = ctx.enter_context(tc.tile_pool(name="ps_g", bufs=1,
                                              space="PSUM"))
        ps_a1 = ctx.enter_context(tc.tile_pool(name="ps_a1", bufs=1,
                                               space="PSUM"))
        ps_a = ctx.enter_context(tc.tile_pool(name="ps_a", bufs=1,
                                              space="PSUM"))
        ps_o = ctx.enter_context(tc.tile_pool(name="ps_o", bufs=2,
                                              space="PSUM"))

        # ---- load constants ----
        c_t = {}
        for name, (shape, dt) in CONST_SPECS.items():
            c_t[name] = cpool.tile(shape, dt, tag=name, name=name)
            nc.sync.dma_start(c_t[name][:], cst[name][:])
        esmov, lindf, identb = c_t["esmov"], c_t["lindf"], c_t["identb"]
        camt01, camrep = c_t["camt01"], c_t["camrep"]
        sp = c_t["sp"]
        w12a, w2bb = c_t["w12a"], c_t["w2bb"]

        gstate = {"sti": 0, "bti": 0, "r3i": 0}

        def emit_front_a(sl, k):
            """e-s(+t) matmuls and the t partition-scatter for supertile k."""
            nch = sl["sts"][k]
            sc = k * NCH2
            xt_sl = sl["xt"]
            # [126, 8, 22]: per chunk 21 e cols + t col (col 21)
            e_ps = ps_e.tile([RC, NCH2, N_JOINTS + 1], FP32, tag="e_ps")
            for c in range(nch):
                nc.tensor.matmul(e_ps[:, c, :], xt_sl[:, sc + c, :],
                                 esmov[:], start=(c == 0), stop=False,
                                 skip_group_check=True)
            t_sb = pts.tile([RC, NCH2], BF16, tag="t_sb")
            nc.vector.tensor_copy(t_sb[:, 0:nch],
                                  e_ps[:, 0:nch, N_JOINTS])
            # R3[q, jj, c] = t[21q+jj, chunk c]
            R3 = pr3.tile([G_CHUNK, N_JOINTS, NCH2], BF16, tag="R3")
            # walk (q; jj; c) both sides, contiguous c-runs
            nc.gpsimd.dma_start(R3[:, :, 0:nch], t_sb[:, 0:nch])
            sl["st2"][k] = dict(e_ps=e_ps, R3=R3, nch=nch, sc=sc)

        def emit_front_b(sl, k):
            """e-t accumulate, prelu, g matmuls + copies for supertile k."""
            st = sl["st2"][k]
            nch, sc = st["nch"], st["sc"]
            e_ps, R3 = st["e_ps"], st["R3"]
            xt_sl, u_sl = sl["xt"], sl["u"]
            for c in range(nch):
                nc.tensor.matmul(e_ps[:, c, 0:N_JOINTS], lindf[:],
                                 _view(R3[:], c, [[NCH2, N_JOINTS]]),
                                 start=False, stop=True,
                                 skip_group_check=True)
            nc.scalar.activation(u_sl[:, sc:sc + nch, :],
                                 e_ps[:, 0:nch, 0:N_JOINTS],
                                 AF.Prelu, alpha=0.2)
            # g (row-major) per chunk, in halves of 4 for PSUM banking
            g_sb = pg.tile([F, NCH2, F], BF16, tag="g_sb")
            nc.sync.dma_start(
                g_sb[RC:RC + 1, :, :],
                _view(cst["w2brow"][:], 0, [[0, NCH2], [1, F]]))
            for h in range(0, nch, 4):
                nh = min(4, nch - h)
                g_ps = ps_g.tile([RC, 4, F], FP32, tag="g_ps")
                for c in range(nh):
                    nc.tensor.matmul(g_ps[:, c, :], xt_sl[:, sc + h + c, :],
                                     w12a[:], start=True, stop=True)
                if h == 0:
                    nc.scalar.copy(g_sb[0:RC, h:h + nh, :], g_ps[:, 0:nh, :])
                else:
                    nc.vector.tensor_copy(g_sb[0:RC, h:h + nh, :],
                                          g_ps[:, 0:nh, :])
            st["g_sb"] = g_sb
            gstate["sti"] += 1

        def emit_softmax(sl):
            """Slab-wide: exp -> rowsum -> recip -> rcam -> acmp."""
            nchs = sl["nchs"]
            u_sl = sl["u"]
            E = pe_.tile([RC, CH_SLAB, N_JOINTS], BF16, tag="E")
            nc.scalar.activation(E[:, 0:nchs, :], u_sl[:, 0:nchs, :], AF.Exp)
            rsum = prs.tile([RC, CH_SLAB], FP32, tag="rsum")
            nc.vector.tensor_reduce(rsum[:, 0:nchs], E[:, 0:nchs, :],
                                    mybir.AxisListType.X, ALU.add)
            rinv = pri.tile([RC, CH_SLAB], FP32, tag="rinv")
            nc.vector.reciprocal(rinv[:, 0:nchs], rsum[:, 0:nchs])
            rcam = prc.tile([RC, CH_SLAB, N_JOINTS], BF16, tag="rcam")
            nc.vector.tensor_tensor(
                rcam[:, 0:nchs, :],
                _view(rinv[:], 0, [[1, nchs], [0, N_JOINTS]]),
                _view(camrep[:], 0, [[0, nchs], [1, N_JOINTS]]),
                ALU.mult)
            acmp = pac.tile([RC, CH_SLAB, N_JOINTS], BF16, tag="acmp")
            nc.vector.tensor_tensor(acmp[:, 0:nchs, :], E[:, 0:nchs, :],
                                    rcam[:, 0:nchs, :], ALU.mult)
            sl["acmp"] = acmp

        def emit_back(sl, k):
            """spread -> gate -> o matmuls -> elu for supertile k."""
            st = sl["st2"][k]
            nch, sc = st["nch"], st["sc"]
            xt_sl, o_sl, acmp = sl["xt"], sl["o"], sl["acmp"]
            g_sb = st["g_sb"]

            # chunk stride padded to 128 so no matmul crosses a psum bank
            at2_ps = ps_a.tile([RC, NCH2, F], FP32, tag="at2")
            for h in range(0, nch, 4):
                nh = min(4, nch - h)
                at1_ps = ps_a1.tile([4 * N_JOINTS, RC], BF16, tag="at1")
                nc.tensor.transpose(at1_ps[0:nh * N_JOINTS, :],
                                    acmp[:, sc + h:sc + h + nh, :],
                                    identb[:])
                atc = patc.tile([4 * N_JOINTS, RC], BF16, tag="atc")
                nc.vector.tensor_copy(atc[0:nh * N_JOINTS, :],
                                      at1_ps[0:nh * N_JOINTS, :])
                for c in range(nh):
                    nc.tensor.matmul(at2_ps[:, h + c, 0:RC],
                                     sp[0:nh * N_JOINTS, c, :],
                                     atc[0:nh * N_JOINTS, :],
                                     start=True, stop=True)

            at = pat.tile([F, NCH2, RC], BF16, tag="at")
            nc.sync.dma_start(
                at[RC:RC + 1, :, :],
                _view(cst["ones126"][:], 0, [[0, NCH2], [1, RC]]))
            nc.vector.tensor_tensor(
                at[0:RC, 0:nch, :], at2_ps[:, 0:nch, 0:RC],
                _view(camt01[:], 0, [[0, nch], [1, RC]]), ALU.mult)

            for h in range(0, nch, 4):
                nh = min(4, nch - h)
                o_ps = ps_o.tile([F, 4, RC], FP32, tag="o_ps")
                nc.tensor.matmul(o_ps[:, 0:nh, 0:RC], w2bb[:],
                                 xt_sl[:, sc + h:sc + h + nh, :],
                                 start=True, stop=False,
                                 skip_group_check=True)
                for c in range(nh):
                    nc.tensor.matmul(o_ps[:, c, 0:RC],
                                     g_sb[0:RC + 1, h + c, :],
                                     at[0:RC + 1, h + c, :],
                                     start=False, stop=True,
                                     skip_group_check=True)
                em = pem.tile([F, 4, RC], BF16, tag="em")
                rr = prr.tile([F, 4, RC], BF16, tag="rr")
                nc.scalar.activation(em[:, 0:nh, :], o_ps[:, 0:nh, 0:RC],
                                     AF.Exp)
                nc.scalar.activation(rr[:, 0:nh, :], o_ps[:, 0:nh, 0:RC],
                                     AF.Relu)
                nc.vector.scalar_tensor_tensor(
                    o_sl[:, sc + h:sc + h + nh, :], em[:, 0:nh, :], -1.0,
                    rr[:, 0:nh, :], op0=ALU.add, op1=ALU.min)

        def setup_slab(sts, c0):
            nchs = sum(sts)
            xt_sl = pxt.tile([F, CH_SLAB, RC], BF16, tag="xt_sl")
            nc.sync.dma_start(xt_sl[:, 0:nchs, :], x_d[:, c0:c0 + nchs, :])
            o_sl = pout.tile([F, CH_SLAB, RC], BF16, tag="o_sl")
            u_sl = pu.tile([RC, CH_SLAB, N_JOINTS], FP32, tag="u")
            return dict(sts=sts, c0=c0, nchs=nchs, xt=xt_sl, o=o_sl,
                        u=u_sl, st2={}, acmp=None)

        def emit_store(sl):
            nc.sync.dma_start(out_d[:, sl["c0"]:sl["c0"] + sl["nchs"], :],
                              sl["o"][:, 0:sl["nchs"], :])

        prev = None
        c0 = 0
        for sts in plan:
            cur = setup_slab(sts, c0)
            c0 += cur["nchs"]
            if prev is not None:
                for k in range(len(prev["sts"])):
                    emit_back(prev, k)
                emit_store(prev)
            for k in range(len(sts)):
                emit_front_a(cur, k)
                if k:
                    emit_front_b(cur, k - 1)
            emit_front_b(cur, len(sts) - 1)
            emit_softmax(cur)
            prev = cur
        for k in range(len(prev["sts"])):
            emit_back(prev, k)
        emit_store(prev)


# ---------------------------------------------------------------------------
_PROG_CACHE = {}


def _get_program(ncht):
    if ncht not in _PROG_CACHE:
        _PROG_CACHE[ncht] = build_program(ncht)
    return _PROG_CACHE[ncht]


def pack_x(x, n_cores=N_CORES, ncht=NCHT):
    """[rows, F] fp32 -> per-core [F, ncht, RC] bf16 (f-major, padded)."""
    x = np.asarray(x, np.float32)
    rows_core = x.shape[0] // n_cores
    xp = np.zeros((n_cores, ncht * RC, F), np.float32)
    xp[:, :rows_core] = x.reshape(n_cores, rows_core, F)
    xp = xp.reshape(n_cores, ncht, RC, F).transpose(0, 3, 1, 2)
    return np.ascontiguousarray(xp.astype(ml_dtypes.bfloat16))


def unpack_out(res_list, rows_core=ROWS_CORE, ncht=NCHT):
    """per-core [F, ncht, RC] bf16 -> [rows, F] fp32."""
    o = np.stack([np.asarray(r, np.float32) for r in res_list])
    o = o.transpose(0, 2, 3, 1).reshape(len(res_list), ncht * RC, F)
    return np.ascontiguousarray(o[:, :rows_core]).reshape(-1, F)


def make_in_maps(x, cam, W1, a, W2_w, W2_b):
    consts = host_consts(cam, W1, a, W2_w, W2_b)
    xp = pack_x(x)
    maps = []
    for core in range(N_CORES):
        m = {"x": xp[core]}
        m.update(consts)
        maps.append(m)
    return maps


def kernel(x, cam, W1, a, W2_w, W2_b):
    from concourse.bass_utils import run_bass_kernel_spmd

    nc = _get_program(NCHT)
    in_maps = make_in_maps(x, cam, W1, a, W2_w, W2_b)
    res = run_bass_kernel_spmd(nc, in_maps, list(range(N_CORES)))
    return unpack_out([res.results[i]["out"] for i in range(N_CORES)])
